# revision 6
# baseline (speedup 1.0000x reference)
"""XNOR-Net conv2d kernel for Trainium2.

Computes conv2d(sign(x), sign(W), stride=1, pad=1) * alpha for
x:(32,256,56,56) f32, W:(256,256,3,3) f32, alpha:(256,1,1) f32.

Strategy: data-parallel over batch (4 images per core x 8 cores).
Per core, implicit GEMM on the PE array in fp8 (sign values +-1 are
exact in fp8e4; accumulation is fp32 in PSUM and all sums are small
integers, so the result is bit-exact vs the f32 reference).

sign(x) lives in SBUF as a zero-padded fp8 image
[128 part = C_in%128, 2 c-groups, 58 rows, 64 row-stride]. Each 3x3
tap is one DoubleRow matmul contracting all 256 input channels
(K = 128 partitions x 2 c-groups): lhsT [128, 2, 128co], rhs
[128, 2, 8, 56] (shifted window, N=448). 9 taps accumulate into one
PSUM bank; copyback applies alpha. x is loaded and signed in 8-row
chunks so the first matmuls start early; output stores go out on the
Activation HWDGE queue to keep the Sync queue free for loads.
"""

import sys

sys.path.insert(0, "/opt/trn_rl_repo")

import numpy as np

import concourse.bass as bass
import concourse.mybir as mybir
from concourse import bacc
from concourse.bass_utils import run_bass_kernel_spmd
from concourse.masks import make_identity
from concourse.tile import TileContext

P = 128
N_CORES = 8
N_IMG = 32
IMG_PER_CORE = N_IMG // N_CORES
C = 256
H = W = 56
HP = 58  # padded rows (0..57)
WS = 64  # row stride of padded buffer (cols 0..57 used, 58+ never read)
CHUNK = 8  # output rows per matmul tile -> N = 8*56 = 448
FP8 = mybir.dt.float8e4

last_result = None  # stash of BassKernelResults for test harnesses


def build_conv_kernel():
    nc = bacc.Bacc()
    x_in = nc.declare_dram_parameter(
        "x", [IMG_PER_CORE, C, H, W], mybir.dt.float32, isOutput=False
    )
    w_in = nc.declare_dram_parameter("w", [C, C, 3, 3], mybir.dt.float32, isOutput=False)
    a_in = nc.declare_dram_parameter("alpha", [C, 1, 1], mybir.dt.float32, isOutput=False)
    y_out = nc.declare_dram_parameter(
        "y", [IMG_PER_CORE, C, H, W], mybir.dt.float32, isOutput=True
    )
    x_ap, w_ap, a_ap, y_ap = x_in[:], w_in[:], a_in[:], y_out[:]

    with TileContext(nc) as tc:
        with (
            tc.tile_pool(name="wpool", bufs=1) as wpool,
            tc.tile_pool(name="xpool", bufs=2) as xpool,
            tc.tile_pool(name="opool", bufs=4) as opool,
            tc.tile_pool(name="pp", bufs=4, space="PSUM") as pp,
        ):
            # ---- one-time weight prep ----
            ident = wpool.tile([P, P], mybir.dt.bfloat16, name="ident")
            make_identity(nc, ident)
            alpha_sb = wpool.tile([P, 2], mybir.dt.float32, name="alpha_sb")
            nc.sync.dma_start(
                out=alpha_sb, in_=a_ap.flatten().rearrange("(mt co) -> co mt", co=P)
            )
            # [ci_lo, cg, mt, pos, co]
            w_lhsT = wpool.tile([P, 2, 2, 9, P], FP8, name="w_lhsT")
            for mt in range(2):
                wsrc = wpool.tile([P, C, 9], mybir.dt.float32, name="wsrc", bufs=2)
                nc.sync.dma_start(
                    out=wsrc,
                    in_=w_ap[mt * P : (mt + 1) * P].rearrange("co ci kh kw -> co ci (kh kw)"),
                )
                wsgn = wpool.tile([P, C, 9], mybir.dt.bfloat16, name="wsgn", bufs=2)
                nc.scalar.sign(wsgn, wsrc)
                for cg in range(2):
                    for pos in range(9):
                        tp = pp.tile([P, P], mybir.dt.bfloat16, name="tp", bufs=2)
                        nc.tensor.transpose(tp, wsgn[:, cg * P : (cg + 1) * P, pos], ident)
                        nc.vector.tensor_copy(out=w_lhsT[:, cg, mt, pos, :], in_=tp)

            # ---- main loop over images ----
            for img in range(IMG_PER_CORE):
                xpad = xpool.tile([P, 2, HP, WS], FP8, name="xpad")
                nc.vector.memset(xpad[:, :, 0, 0:58], 0.0)
                nc.vector.memset(xpad[:, :, HP - 1, 0:58], 0.0)
                nc.vector.memset(xpad[:, :, 1 : HP - 1, 0], 0.0)
                nc.vector.memset(xpad[:, :, 1 : HP - 1, 57], 0.0)
                # load + sign in 8-row chunks so matmuls can start early
                for cg in range(2):
                    for r0 in range(0, H, CHUNK):
                        xsrc = xpool.tile([P, CHUNK, W], mybir.dt.float32, name="xsrc")
                        nc.sync.dma_start(
                            out=xsrc, in_=x_ap[img, cg * P : (cg + 1) * P, r0 : r0 + CHUNK]
                        )
                        nc.scalar.sign(
                            xpad[:, cg, r0 + 1 : r0 + 1 + CHUNK, 1 : W + 1], xsrc
                        )

                for h0 in range(0, H, CHUNK):
                    for mt in range(2):
                        acc = pp.tile([P, CHUNK * W], mybir.dt.float32, name="acc")
                        k = 0
                        for kh in range(3):
                            for kw in range(3):
                                nc.tensor.matmul(
                                    acc,
                                    w_lhsT[:, :, mt, kh * 3 + kw, :],
                                    xpad[:, :, h0 + kh : h0 + kh + CHUNK, kw : kw + W],
                                    start=(k == 0),
                                    stop=(k == 8),
                                    perf_mode=mybir.MatmulPerfMode.DoubleRow,
                                )
                                k += 1
                        ot = opool.tile([P, CHUNK, W], mybir.dt.float32, name="ot")
                        nc.vector.tensor_scalar_mul(
                            out=ot,
                            in0=acc.rearrange("p (r c) -> p r c", c=W),
                            scalar1=alpha_sb[:, mt : mt + 1],
                        )
                        nc.scalar.dma_start(
                            out=y_ap[img, mt * P : (mt + 1) * P, h0 : h0 + CHUNK, :],
                            in_=ot,
                        )
    nc.compile()
    return nc


def kernel(x, weight, alpha, trace=False):
    global last_result
    x = np.ascontiguousarray(np.asarray(x, dtype=np.float32))
    weight = np.ascontiguousarray(np.asarray(weight, dtype=np.float32))
    alpha = np.ascontiguousarray(np.asarray(alpha, dtype=np.float32))

    nc = build_conv_kernel()
    in_maps = [
        {
            "x": np.ascontiguousarray(x[i * IMG_PER_CORE : (i + 1) * IMG_PER_CORE]),
            "w": weight,
            "alpha": alpha,
        }
        for i in range(N_CORES)
    ]
    res = run_bass_kernel_spmd(nc, in_maps, list(range(N_CORES)), trace=trace)
    last_result = res
    out = np.concatenate([res.results[i]["y"] for i in range(N_CORES)], axis=0)
    return out.astype(np.float32, copy=False)


# revision 7
# speedup vs baseline: 1.0067x; 1.0067x over previous
"""XNOR-Net conv2d kernel for Trainium2.

Computes conv2d(sign(x), sign(W), stride=1, pad=1) * alpha for
x:(32,256,56,56) f32, W:(256,256,3,3) f32, alpha:(256,1,1) f32.

Strategy: data-parallel over batch (4 images per core x 8 cores).
Per core, implicit GEMM on the PE array in fp8 (sign values +-1 are
exact in fp8e4; accumulation is fp32 in PSUM and all sums are small
integers, so the result is bit-exact vs the f32 reference).

sign(x) lives in SBUF as a zero-padded fp8 image
[128 part = C_in%128, 2 c-groups, 58 rows, 64 row-stride]. Each 3x3
tap is one DoubleRowSwInterleave matmul contracting all 256 input
channels (K = 128 partitions x 2 c-groups): the weights are stored
pre-interleaved ([A127,B127,A126,B126,...] per ci row, produced on
chip by a flip-permutation PE transpose + stride-2 copyback), so
LDWEIGHTS streams contiguously; rhs is a shifted 4D window
[128, 2cg, 8 rows, 56 cols] (N=448). 9 taps accumulate into one PSUM
bank; copyback applies alpha. x is loaded + signed in 8-row chunks so
first matmuls start early.
"""

import sys

sys.path.insert(0, "/opt/trn_rl_repo")

import numpy as np

import concourse.bass as bass
import concourse.mybir as mybir
from concourse import bacc
from concourse.bass_utils import run_bass_kernel_spmd
from concourse.tile import TileContext

P = 128
N_CORES = 8
N_IMG = 32
IMG_PER_CORE = N_IMG // N_CORES
C = 256
H = W = 56
HP = 58  # padded rows (0..57)
WS = 64  # row stride of padded buffer (cols 0..57 used, 58+ never read)
CHUNK = 8  # output rows per matmul tile -> N = 8*56 = 448
FP8 = mybir.dt.float8e4

last_result = None  # stash of BassKernelResults for test harnesses


def make_flip(nc, flip):
    """flip[x, y] = 1 iff x + y == 127 (anti-diagonal permutation)."""
    sq = flip.shape[0]
    nc.gpsimd.memset(flip, 0.0)
    nc.gpsimd.affine_select(
        out=flip,
        in_=flip,
        compare_op=mybir.AluOpType.not_equal,
        fill=1.0,
        base=-(sq - 1),
        pattern=[[1, sq]],
        channel_multiplier=1,
    )


def build_conv_kernel():
    nc = bacc.Bacc()
    x_in = nc.declare_dram_parameter(
        "x", [IMG_PER_CORE, C, H, W], mybir.dt.float32, isOutput=False
    )
    w_in = nc.declare_dram_parameter("w", [C, C, 3, 3], mybir.dt.float32, isOutput=False)
    a_in = nc.declare_dram_parameter("alpha", [C, 1, 1], mybir.dt.float32, isOutput=False)
    y_out = nc.declare_dram_parameter(
        "y", [IMG_PER_CORE, C, H, W], mybir.dt.float32, isOutput=True
    )
    x_ap, w_ap, a_ap, y_ap = x_in[:], w_in[:], a_in[:], y_out[:]

    with TileContext(nc) as tc:
        with (
            tc.tile_pool(name="wpool", bufs=1) as wpool,
            tc.tile_pool(name="xpool", bufs=2) as xpool,
            tc.tile_pool(name="opool", bufs=4) as opool,
            tc.tile_pool(name="pp", bufs=4, space="PSUM") as pp,
        ):
            # warm up the ACT function table while the first DMAs run
            warm = wpool.tile([P, 1], mybir.dt.float32, name="warm")
            nc.vector.memset(warm, 0.0)
            nc.scalar.sign(warm, warm)

            flip = wpool.tile([P, P], mybir.dt.bfloat16, name="flip")
            make_flip(nc, flip)
            alpha_sb = wpool.tile([P, 2], mybir.dt.float32, name="alpha_sb")
            nc.sync.dma_start(
                out=alpha_sb, in_=a_ap.flatten().rearrange("(mt co) -> co mt", co=P)
            )
            # interleaved weights: [ci, mt, pos, co_rev (step 2), cg]
            w_swi = wpool.tile([P, 2, 9, P, 2], FP8, name="w_swi")
            for mt in range(2):
                wsrc = wpool.tile([P, C, 9], mybir.dt.float32, name="wsrc", bufs=2)
                nc.sync.dma_start(
                    out=wsrc,
                    in_=w_ap[mt * P : (mt + 1) * P].rearrange("co ci kh kw -> co ci (kh kw)"),
                )
                wsgn = wpool.tile([P, C, 9], mybir.dt.bfloat16, name="wsgn", bufs=2)
                nc.scalar.sign(wsgn, wsrc)
                for cg in range(2):
                    for pos in range(9):
                        tp = pp.tile([P, P], mybir.dt.bfloat16, name="tp", bufs=2)
                        # tp[ci, j] = sign(W)[co=127-j, ci]  (flip-transpose)
                        nc.tensor.transpose(tp, wsgn[:, cg * P : (cg + 1) * P, pos], flip)
                        nc.vector.tensor_copy(out=w_swi[:, mt, pos, :, cg], in_=tp)

            # ---- main loop over images ----
            for img in range(IMG_PER_CORE):
                xpad = xpool.tile([P, 2, HP, WS], FP8, name="xpad")
                nc.vector.memset(xpad[:, :, 0, 0:58], 0.0)
                nc.vector.memset(xpad[:, :, HP - 1, 0:58], 0.0)
                nc.vector.memset(xpad[:, :, 1 : HP - 1, 0], 0.0)
                nc.vector.memset(xpad[:, :, 1 : HP - 1, 57], 0.0)
                # load + sign in 8-row chunks so matmuls can start early
                for cg in range(2):
                    for r0 in range(0, H, CHUNK):
                        xsrc = xpool.tile([P, CHUNK, W], mybir.dt.float32, name="xsrc")
                        nc.sync.dma_start(
                            out=xsrc, in_=x_ap[img, cg * P : (cg + 1) * P, r0 : r0 + CHUNK]
                        )
                        nc.scalar.sign(
                            xpad[:, cg, r0 + 1 : r0 + 1 + CHUNK, 1 : W + 1], xsrc
                        )

                for h0 in range(0, H, CHUNK):
                    for mt in range(2):
                        acc = pp.tile([P, CHUNK * W], mybir.dt.float32, name="acc")
                        k = 0
                        for kh in range(3):
                            for kw in range(3):
                                lhsT = (
                                    w_swi[:, mt, kh * 3 + kw]
                                    .rearrange("p c t -> p (c t)")
                                    .rearrange("p (two f) -> p two f", two=2)
                                )
                                nc.tensor.matmul(
                                    acc,
                                    lhsT,
                                    xpad[:, :, h0 + kh : h0 + kh + CHUNK, kw : kw + W],
                                    start=(k == 0),
                                    stop=(k == 8),
                                    perf_mode=mybir.MatmulPerfMode.DoubleRowSwInterleave,
                                )
                                k += 1
                        ot = opool.tile([P, CHUNK, W], mybir.dt.float32, name="ot")
                        nc.vector.tensor_scalar_mul(
                            out=ot,
                            in0=acc.rearrange("p (r c) -> p r c", c=W),
                            scalar1=alpha_sb[:, mt : mt + 1],
                        )
                        nc.sync.dma_start(
                            out=y_ap[img, mt * P : (mt + 1) * P, h0 : h0 + CHUNK, :],
                            in_=ot,
                        )
    nc.compile()
    return nc


def kernel(x, weight, alpha, trace=False):
    global last_result
    x = np.ascontiguousarray(np.asarray(x, dtype=np.float32))
    weight = np.ascontiguousarray(np.asarray(weight, dtype=np.float32))
    alpha = np.ascontiguousarray(np.asarray(alpha, dtype=np.float32))

    nc = build_conv_kernel()
    in_maps = [
        {
            "x": np.ascontiguousarray(x[i * IMG_PER_CORE : (i + 1) * IMG_PER_CORE]),
            "w": weight,
            "alpha": alpha,
        }
        for i in range(N_CORES)
    ]
    res = run_bass_kernel_spmd(nc, in_maps, list(range(N_CORES)), trace=trace)
    last_result = res
    out = np.concatenate([res.results[i]["y"] for i in range(N_CORES)], axis=0)
    return out.astype(np.float32, copy=False)


# revision 8
# speedup vs baseline: 1.2807x; 1.2721x over previous
"""XNOR-Net conv2d kernel for Trainium2.

Computes conv2d(sign(x), sign(W), stride=1, pad=1) * alpha for
x:(32,256,56,56) f32, W:(256,256,3,3) f32, alpha:(256,1,1) f32.

Strategy: data-parallel over batch (4 images per core x 8 cores).
Per core, implicit GEMM on the PE array in fp8 (sign values +-1 are
exact in fp8e4; accumulation is fp32 in PSUM and all sums are small
integers, so the result is bit-exact vs the f32 reference).

sign(x) lives in SBUF as a zero-padded fp8 image
[128 part = C_in%128, 2 c-groups, 58 rows, 64 row-stride]. Each 3x3
tap is one DoubleRow matmul contracting all 256 input channels
(K = 128 partitions x 2 c-groups): lhsT [128, 2cg, 128co], rhs
[128, 2cg, 8 rows, 56 cols] (shifted window, N=448). 9 taps
accumulate into one PSUM bank; copyback applies alpha.

Pipelining: x is loaded + signed in 8-row chunks with an 8-deep xsrc
ring so DMA completion latency stays hidden across image boundaries;
weight DMAs ride the Activation HWDGE queue so x loads start
immediately on the Sync queue.
"""

import sys

sys.path.insert(0, "/opt/trn_rl_repo")

import numpy as np

import concourse.bass as bass
import concourse.mybir as mybir
from concourse import bacc
from concourse.bass_utils import run_bass_kernel_spmd
from concourse.masks import make_identity
from concourse.tile import TileContext

P = 128
N_CORES = 8
N_IMG = 32
IMG_PER_CORE = N_IMG // N_CORES
C = 256
H = W = 56
HP = 58  # padded rows (0..57)
WS = 64  # row stride of padded buffer (cols 0..57 used, 58+ never read)
CHUNK = 8  # output rows per matmul tile -> N = 8*56 = 448
FP8 = mybir.dt.float8e4

last_result = None  # stash of BassKernelResults for test harnesses


def build_conv_kernel():
    nc = bacc.Bacc()
    x_in = nc.declare_dram_parameter(
        "x", [IMG_PER_CORE, C, H, W], mybir.dt.float32, isOutput=False
    )
    w_in = nc.declare_dram_parameter("w", [C, C, 3, 3], mybir.dt.float32, isOutput=False)
    a_in = nc.declare_dram_parameter("alpha", [C, 1, 1], mybir.dt.float32, isOutput=False)
    y_out = nc.declare_dram_parameter(
        "y", [IMG_PER_CORE, C, H, W], mybir.dt.float32, isOutput=True
    )
    x_ap, w_ap, a_ap, y_ap = x_in[:], w_in[:], a_in[:], y_out[:]

    with TileContext(nc) as tc:
        with (
            tc.tile_pool(name="wpool", bufs=1) as wpool,
            tc.tile_pool(name="xpool", bufs=2) as xpool,
            tc.tile_pool(name="opool", bufs=4) as opool,
            tc.tile_pool(name="pp", bufs=4, space="PSUM") as pp,
        ):
            # warm up the ACT function table while the first DMAs run
            warm = wpool.tile([P, 1], mybir.dt.float32, name="warm")
            nc.vector.memset(warm, 0.0)
            nc.scalar.sign(warm, warm)

            ident = wpool.tile([P, P], mybir.dt.bfloat16, name="ident")
            make_identity(nc, ident)
            alpha_sb = wpool.tile([P, 2], mybir.dt.float32, name="alpha_sb")
            nc.sync.dma_start(
                out=alpha_sb, in_=a_ap.flatten().rearrange("(mt co) -> co mt", co=P)
            )
            # [ci_lo, cg, mt, pos, co]
            w_lhsT = wpool.tile([P, 2, 2, 9, P], FP8, name="w_lhsT")
            for mt in range(2):
                wsrc = wpool.tile([P, C, 9], mybir.dt.float32, name="wsrc", bufs=2)
                # ride the ACT HWDGE queue: keeps the Sync queue free for x
                nc.scalar.dma_start(
                    out=wsrc,
                    in_=w_ap[mt * P : (mt + 1) * P].rearrange("co ci kh kw -> co ci (kh kw)"),
                )
                wsgn = wpool.tile([P, C, 9], mybir.dt.bfloat16, name="wsgn", bufs=2)
                nc.scalar.sign(wsgn, wsrc)
                for cg in range(2):
                    for pos in range(9):
                        tp = pp.tile([P, P], mybir.dt.bfloat16, name="tp", bufs=2)
                        nc.tensor.transpose(tp, wsgn[:, cg * P : (cg + 1) * P, pos], ident)
                        nc.vector.tensor_copy(out=w_lhsT[:, cg, mt, pos, :], in_=tp)

            # ---- main loop over images ----
            for img in range(IMG_PER_CORE):
                xpad = xpool.tile([P, 2, HP, WS], FP8, name="xpad")
                nc.vector.memset(xpad[:, :, 0, 0:58], 0.0)
                nc.vector.memset(xpad[:, :, HP - 1, 0:58], 0.0)
                nc.vector.memset(xpad[:, :, 1 : HP - 1, 0], 0.0)
                nc.vector.memset(xpad[:, :, 1 : HP - 1, 57], 0.0)
                # load + sign in 8-row chunks; deep ring hides DMA latency
                for r0 in range(0, H, CHUNK):
                    for cg in range(2):
                        xsrc = xpool.tile(
                            [P, CHUNK, W], mybir.dt.float32, name="xsrc", bufs=8
                        )
                        nc.sync.dma_start(
                            out=xsrc, in_=x_ap[img, cg * P : (cg + 1) * P, r0 : r0 + CHUNK]
                        )
                        nc.scalar.sign(
                            xpad[:, cg, r0 + 1 : r0 + 1 + CHUNK, 1 : W + 1], xsrc
                        )

                for h0 in range(0, H, CHUNK):
                    for mt in range(2):
                        acc = pp.tile([P, CHUNK * W], mybir.dt.float32, name="acc")
                        k = 0
                        for kh in range(3):
                            for kw in range(3):
                                nc.tensor.matmul(
                                    acc,
                                    w_lhsT[:, :, mt, kh * 3 + kw, :],
                                    xpad[:, :, h0 + kh : h0 + kh + CHUNK, kw : kw + W],
                                    start=(k == 0),
                                    stop=(k == 8),
                                    perf_mode=mybir.MatmulPerfMode.DoubleRow,
                                )
                                k += 1
                        ot = opool.tile([P, CHUNK, W], mybir.dt.float32, name="ot")
                        nc.vector.tensor_scalar_mul(
                            out=ot,
                            in0=acc.rearrange("p (r c) -> p r c", c=W),
                            scalar1=alpha_sb[:, mt : mt + 1],
                        )
                        nc.sync.dma_start(
                            out=y_ap[img, mt * P : (mt + 1) * P, h0 : h0 + CHUNK, :],
                            in_=ot,
                        )
    nc.compile()
    return nc


def kernel(x, weight, alpha, trace=False):
    global last_result
    x = np.ascontiguousarray(np.asarray(x, dtype=np.float32))
    weight = np.ascontiguousarray(np.asarray(weight, dtype=np.float32))
    alpha = np.ascontiguousarray(np.asarray(alpha, dtype=np.float32))

    nc = build_conv_kernel()
    in_maps = [
        {
            "x": np.ascontiguousarray(x[i * IMG_PER_CORE : (i + 1) * IMG_PER_CORE]),
            "w": weight,
            "alpha": alpha,
        }
        for i in range(N_CORES)
    ]
    res = run_bass_kernel_spmd(nc, in_maps, list(range(N_CORES)), trace=trace)
    last_result = res
    out = np.concatenate([res.results[i]["y"] for i in range(N_CORES)], axis=0)
    return out.astype(np.float32, copy=False)


# revision 10
# speedup vs baseline: 1.2840x; 1.0026x over previous
"""XNOR-Net conv2d kernel for Trainium2.

Computes conv2d(sign(x), sign(W), stride=1, pad=1) * alpha for
x:(32,256,56,56) f32, W:(256,256,3,3) f32, alpha:(256,1,1) f32.

Strategy: data-parallel over batch (4 images per core x 8 cores).
Per core, implicit GEMM on the PE array in fp8. sign(x) is +-1 in
fp8e4 (exact); sign(W) is represented as +-0.5 (one-pass DVE compute:
(w>0) - 0.5), with the missing x2 folded into alpha. Products are
+-0.5, accumulated in fp32 PSUM -> half-integers, exact; the final
scale restores integers, so the result is bit-exact vs the reference.

sign(x) lives in SBUF as a zero-padded fp8 image
[128 part = C_in%128, 2 c-groups, 58 rows, 64 row-stride]. Each 3x3
tap is one DoubleRow matmul contracting all 256 input channels
(K = 128 partitions x 2 c-groups): lhsT [128, 2cg, 128co], rhs
[128, 2cg, 8 rows, 56 cols] (shifted window, N=448). 9 taps
accumulate into one PSUM bank; copyback applies 2*alpha.

Pipelining: x is loaded + signed in 8-row chunks with a 14-deep xsrc
ring (hides HWDGE completion latency, ~2.4us); weight DMAs ride the
Activation HWDGE queue in 4 (mt x cg) pieces and are signed on DVE so
the ACT queue only ever holds x signs.
"""

import sys

sys.path.insert(0, "/opt/trn_rl_repo")

import numpy as np

import concourse.bass as bass
import concourse.mybir as mybir
from concourse import bacc
from concourse.bass_utils import run_bass_kernel_spmd
from concourse.masks import make_identity
from concourse.tile import TileContext

P = 128
N_CORES = 8
N_IMG = 32
IMG_PER_CORE = N_IMG // N_CORES
C = 256
H = W = 56
HP = 58  # padded rows (0..57)
WS = 64  # row stride of padded buffer (cols 0..57 used, 58+ never read)
CHUNK = 8  # output rows per matmul tile -> N = 8*56 = 448
FP8 = mybir.dt.float8e4

last_result = None  # stash of BassKernelResults for test harnesses


def build_conv_kernel():
    nc = bacc.Bacc()
    x_in = nc.declare_dram_parameter(
        "x", [IMG_PER_CORE, C, H, W], mybir.dt.float32, isOutput=False
    )
    w_in = nc.declare_dram_parameter("w", [C, C, 3, 3], mybir.dt.float32, isOutput=False)
    a_in = nc.declare_dram_parameter("alpha", [C, 1, 1], mybir.dt.float32, isOutput=False)
    y_out = nc.declare_dram_parameter(
        "y", [IMG_PER_CORE, C, H, W], mybir.dt.float32, isOutput=True
    )
    x_ap, w_ap, a_ap, y_ap = x_in[:], w_in[:], a_in[:], y_out[:]

    with TileContext(nc) as tc:
        with (
            tc.tile_pool(name="wpool", bufs=1) as wpool,
            tc.tile_pool(name="xpool", bufs=2) as xpool,
            tc.tile_pool(name="opool", bufs=6) as opool,
            tc.tile_pool(name="pp", bufs=4, space="PSUM") as pp,
        ):
            # warm up the ACT function table while the first DMAs run
            warm = wpool.tile([P, 1], mybir.dt.float32, name="warm")
            nc.vector.memset(warm, 0.0)
            nc.scalar.sign(warm, warm)

            ident = wpool.tile([P, P], mybir.dt.bfloat16, name="ident")
            make_identity(nc, ident)
            alpha_sb = wpool.tile([P, 2], mybir.dt.float32, name="alpha_sb")
            nc.sync.dma_start(
                out=alpha_sb, in_=a_ap.flatten().rearrange("(mt co) -> co mt", co=P)
            )
            # weights carry +-0.5; restore the factor 2 here (DVE: keeps
            # the ACT queue free for x signs)
            nc.vector.tensor_scalar(
                out=alpha_sb,
                in0=alpha_sb,
                scalar1=2.0,
                scalar2=None,
                op0=mybir.AluOpType.mult,
            )

            # [ci_lo, cg, mt, pos, co]
            w_lhsT = wpool.tile([P, 2, 2, 9, P], FP8, name="w_lhsT")
            for mt in range(2):
                for cg in range(2):
                    wsrc = wpool.tile([P, P, 9], mybir.dt.float32, name="wsrc", bufs=2)
                    # ACT HWDGE queue: keeps the Sync queue free for x loads
                    nc.scalar.dma_start(
                        out=wsrc,
                        in_=w_ap[
                            mt * P : (mt + 1) * P, cg * P : (cg + 1) * P
                        ].rearrange("co ci kh kw -> co ci (kh kw)"),
                    )
                    # one-pass half-sign on DVE: (w > 0) - 0.5 -> +-0.5
                    wsgn = wpool.tile([P, P, 9], mybir.dt.bfloat16, name="wsgn", bufs=2)
                    nc.vector.tensor_scalar(
                        out=wsgn,
                        in0=wsrc,
                        scalar1=0.0,
                        scalar2=0.5,
                        op0=mybir.AluOpType.is_gt,
                        op1=mybir.AluOpType.subtract,
                    )
                    for pos in range(9):
                        tp = pp.tile([P, P], mybir.dt.bfloat16, name="tp", bufs=2)
                        nc.tensor.transpose(tp, wsgn[:, :, pos], ident)
                        nc.vector.tensor_copy(out=w_lhsT[:, cg, mt, pos, :], in_=tp)

            # ---- main loop over images ----
            for img in range(IMG_PER_CORE):
                xpad = xpool.tile([P, 2, HP, WS], FP8, name="xpad")
                nc.vector.memset(xpad[:, :, 0, 0:58], 0.0)
                nc.vector.memset(xpad[:, :, HP - 1, 0:58], 0.0)
                nc.vector.memset(xpad[:, :, 1 : HP - 1, 0], 0.0)
                nc.vector.memset(xpad[:, :, 1 : HP - 1, 57], 0.0)
                # load + sign in 8-row chunks; deep ring hides DMA latency
                for r0 in range(0, H, CHUNK):
                    for cg in range(2):
                        xsrc = xpool.tile(
                            [P, CHUNK, W], mybir.dt.float32, name="xsrc", bufs=14
                        )
                        nc.sync.dma_start(
                            out=xsrc, in_=x_ap[img, cg * P : (cg + 1) * P, r0 : r0 + CHUNK]
                        )
                        nc.scalar.sign(
                            xpad[:, cg, r0 + 1 : r0 + 1 + CHUNK, 1 : W + 1], xsrc
                        )

                for h0 in range(0, H, CHUNK):
                    for mt in range(2):
                        acc = pp.tile([P, CHUNK * W], mybir.dt.float32, name="acc")
                        k = 0
                        for kh in range(3):
                            for kw in range(3):
                                nc.tensor.matmul(
                                    acc,
                                    w_lhsT[:, :, mt, kh * 3 + kw, :],
                                    xpad[:, :, h0 + kh : h0 + kh + CHUNK, kw : kw + W],
                                    start=(k == 0),
                                    stop=(k == 8),
                                    perf_mode=mybir.MatmulPerfMode.DoubleRow,
                                )
                                k += 1
                        ot = opool.tile([P, CHUNK, W], mybir.dt.float32, name="ot")
                        nc.vector.tensor_scalar_mul(
                            out=ot,
                            in0=acc.rearrange("p (r c) -> p r c", c=W),
                            scalar1=alpha_sb[:, mt : mt + 1],
                        )
                        nc.sync.dma_start(
                            out=y_ap[img, mt * P : (mt + 1) * P, h0 : h0 + CHUNK, :],
                            in_=ot,
                        )
    nc.compile()
    return nc


def kernel(x, weight, alpha, trace=False):
    global last_result
    x = np.ascontiguousarray(np.asarray(x, dtype=np.float32))
    weight = np.ascontiguousarray(np.asarray(weight, dtype=np.float32))
    alpha = np.ascontiguousarray(np.asarray(alpha, dtype=np.float32))

    nc = build_conv_kernel()
    in_maps = [
        {
            "x": np.ascontiguousarray(x[i * IMG_PER_CORE : (i + 1) * IMG_PER_CORE]),
            "w": weight,
            "alpha": alpha,
        }
        for i in range(N_CORES)
    ]
    res = run_bass_kernel_spmd(nc, in_maps, list(range(N_CORES)), trace=trace)
    last_result = res
    out = np.concatenate([res.results[i]["y"] for i in range(N_CORES)], axis=0)
    return out.astype(np.float32, copy=False)


# revision 12
# speedup vs baseline: 1.3996x; 1.0900x over previous
"""XNOR-Net conv2d kernel for Trainium2.

Computes conv2d(sign(x), sign(W), stride=1, pad=1) * alpha for
x:(32,256,56,56) f32, W:(256,256,3,3) f32, alpha:(256,1,1) f32.

Strategy: data-parallel over batch (4 images per core x 8 cores).
Per core, implicit GEMM on the PE array in fp8. sign(x) is +-1 in
fp8e4 (exact); sign(W) is represented as +-0.5 (one-pass DVE compute:
(w>0) - 0.5), with the missing x2 folded into alpha. Products are
+-0.5, accumulated in fp32 PSUM -> half-integers, exact; the final
scale restores integers, so the result is bit-exact vs the reference.

sign(x) lives in SBUF as a zero-padded fp8 image
[128 part = C_in%128, 2 c-groups, 58 rows, 64 row-stride]. Each 3x3
tap is one DoubleRow matmul contracting all 256 input channels
(K = 128 partitions x 2 c-groups): lhsT [128, 2cg, 128co], rhs
[128, 2cg, 8 rows, 56 cols] (shifted window, N=448). 9 taps
accumulate into one PSUM bank; copyback applies 2*alpha.

Pipelining: software-pipelined emission — image i+1's chunked loads
are emitted before image i's matmul/store phase, so the Sync DMA
queue always has ready loads ahead of copyback-gated stores; signs
are the only ACT work; weights ride the ACT HWDGE queue in 4 pieces
and are signed on DVE.
"""

import sys

sys.path.insert(0, "/opt/trn_rl_repo")

import numpy as np

import concourse.bass as bass
import concourse.mybir as mybir
from concourse import bacc
from concourse.bass_utils import run_bass_kernel_spmd
from concourse.masks import make_identity
from concourse.tile import TileContext

P = 128
N_CORES = 8
N_IMG = 32
IMG_PER_CORE = N_IMG // N_CORES
C = 256
H = W = 56
HP = 58  # padded rows (0..57)
WS = 64  # row stride of padded buffer (cols 0..57 used, 58+ never read)
CHUNK = 8  # output rows per matmul tile -> N = 8*56 = 448
FP8 = mybir.dt.float8e4

last_result = None  # stash of BassKernelResults for test harnesses


def build_conv_kernel():
    nc = bacc.Bacc()
    x_in = nc.declare_dram_parameter(
        "x", [IMG_PER_CORE, C, H, W], mybir.dt.float32, isOutput=False
    )
    w_in = nc.declare_dram_parameter("w", [C, C, 3, 3], mybir.dt.float32, isOutput=False)
    a_in = nc.declare_dram_parameter("alpha", [C, 1, 1], mybir.dt.float32, isOutput=False)
    y_out = nc.declare_dram_parameter(
        "y", [IMG_PER_CORE, C, H, W], mybir.dt.float32, isOutput=True
    )
    x_ap, w_ap, a_ap, y_ap = x_in[:], w_in[:], a_in[:], y_out[:]

    with TileContext(nc) as tc:
        with (
            tc.tile_pool(name="wpool", bufs=1) as wpool,
            tc.tile_pool(name="xpool", bufs=3) as xpool,
            tc.tile_pool(name="opool", bufs=6) as opool,
            tc.tile_pool(name="pp", bufs=4, space="PSUM") as pp,
        ):
            # warm up the ACT function table while the first DMAs run
            warm = wpool.tile([P, 1], mybir.dt.float32, name="warm")
            nc.vector.memset(warm, 0.0)
            nc.scalar.sign(warm, warm)

            ident = wpool.tile([P, P], mybir.dt.bfloat16, name="ident")
            make_identity(nc, ident)
            alpha_sb = wpool.tile([P, 2], mybir.dt.float32, name="alpha_sb")
            nc.sync.dma_start(
                out=alpha_sb, in_=a_ap.flatten().rearrange("(mt co) -> co mt", co=P)
            )
            # weights carry +-0.5; restore the factor 2 here (on DVE so the
            # ACT queue stays free for x signs)
            nc.vector.tensor_scalar(
                out=alpha_sb,
                in0=alpha_sb,
                scalar1=2.0,
                scalar2=None,
                op0=mybir.AluOpType.mult,
            )

            # ---- one-time weight prep: 4 (mt x cg) pieces ----
            # [ci_lo, cg, mt, pos, co]
            w_lhsT = wpool.tile([P, 2, 2, 9, P], FP8, name="w_lhsT")
            for mt in range(2):
                for cg in range(2):
                    wsrc = wpool.tile([P, P, 9], mybir.dt.float32, name="wsrc", bufs=2)
                    # ACT HWDGE queue: keeps the Sync queue free for x loads
                    nc.scalar.dma_start(
                        out=wsrc,
                        in_=w_ap[
                            mt * P : (mt + 1) * P, cg * P : (cg + 1) * P
                        ].rearrange("co ci kh kw -> co ci (kh kw)"),
                    )
                    # one-pass half-sign on DVE: (w > 0) - 0.5 -> +-0.5
                    wsgn = wpool.tile([P, P, 9], mybir.dt.bfloat16, name="wsgn", bufs=2)
                    nc.vector.tensor_scalar(
                        out=wsgn,
                        in0=wsrc,
                        scalar1=0.0,
                        scalar2=0.5,
                        op0=mybir.AluOpType.is_gt,
                        op1=mybir.AluOpType.subtract,
                    )
                    for pos in range(9):
                        tp = pp.tile([P, P], mybir.dt.bfloat16, name="tp", bufs=2)
                        nc.tensor.transpose(tp, wsgn[:, :, pos], ident)
                        nc.vector.tensor_copy(out=w_lhsT[:, cg, mt, pos, :], in_=tp)

            # ---- software-pipelined main loop ----
            xpads = {}

            def emit_loads(img):
                xpad = xpool.tile([P, 2, HP, WS], FP8, name="xpad")
                xpads[img] = xpad
                nc.vector.memset(xpad[:, :, 0, 0:58], 0.0)
                nc.vector.memset(xpad[:, :, HP - 1, 0:58], 0.0)
                nc.vector.memset(xpad[:, :, 1 : HP - 1, 0], 0.0)
                nc.vector.memset(xpad[:, :, 1 : HP - 1, 57], 0.0)
                srcs = []
                for r0 in range(0, H, CHUNK):
                    for cg in range(2):
                        xsrc = xpool.tile(
                            [P, CHUNK, W], mybir.dt.float32, name="xsrc", bufs=14
                        )
                        nc.sync.dma_start(
                            out=xsrc,
                            in_=x_ap[img, cg * P : (cg + 1) * P, r0 : r0 + CHUNK],
                        )
                        srcs.append((r0, cg, xsrc))
                return srcs

            def emit_signs(img, srcs):
                xpad = xpads[img]
                for r0, cg, xsrc in srcs:
                    nc.scalar.sign(
                        xpad[:, cg, r0 + 1 : r0 + 1 + CHUNK, 1 : W + 1], xsrc
                    )

            def emit_mms(img):
                xpad = xpads[img]
                for h0 in range(0, H, CHUNK):
                    for mt in range(2):
                        acc = pp.tile([P, CHUNK * W], mybir.dt.float32, name="acc")
                        k = 0
                        for kh in range(3):
                            for kw in range(3):
                                nc.tensor.matmul(
                                    acc,
                                    w_lhsT[:, :, mt, kh * 3 + kw, :],
                                    xpad[:, :, h0 + kh : h0 + kh + CHUNK, kw : kw + W],
                                    start=(k == 0),
                                    stop=(k == 8),
                                    perf_mode=mybir.MatmulPerfMode.DoubleRow,
                                )
                                k += 1
                        ot = opool.tile([P, CHUNK, W], mybir.dt.float32, name="ot")
                        nc.vector.tensor_scalar_mul(
                            out=ot,
                            in0=acc.rearrange("p (r c) -> p r c", c=W),
                            scalar1=alpha_sb[:, mt : mt + 1],
                        )
                        nc.sync.dma_start(
                            out=y_ap[img, mt * P : (mt + 1) * P, h0 : h0 + CHUNK, :],
                            in_=ot,
                        )

            # two-image lookahead: loads+signs of img i+1/i+2 are emitted
            # before matmuls of img i (xpad bufs=3 makes slot i+2 fresh)
            for img in (0, 1):
                srcs = emit_loads(img)
                emit_signs(img, srcs)
            for img in range(IMG_PER_CORE):
                emit_mms(img)
                if img + 2 < IMG_PER_CORE:
                    srcs = emit_loads(img + 2)
                    emit_signs(img + 2, srcs)
    nc.compile()
    return nc


def kernel(x, weight, alpha, trace=False):
    global last_result
    x = np.ascontiguousarray(np.asarray(x, dtype=np.float32))
    weight = np.ascontiguousarray(np.asarray(weight, dtype=np.float32))
    alpha = np.ascontiguousarray(np.asarray(alpha, dtype=np.float32))

    nc = build_conv_kernel()
    in_maps = [
        {
            "x": np.ascontiguousarray(x[i * IMG_PER_CORE : (i + 1) * IMG_PER_CORE]),
            "w": weight,
            "alpha": alpha,
        }
        for i in range(N_CORES)
    ]
    res = run_bass_kernel_spmd(nc, in_maps, list(range(N_CORES)), trace=trace)
    last_result = res
    out = np.concatenate([res.results[i]["y"] for i in range(N_CORES)], axis=0)
    return out.astype(np.float32, copy=False)
